# revision 7
# baseline (speedup 1.0000x reference)
"""Distributed Trainium2 Bass kernel for nn_AIGGenerator (GCN encode + masked
top-2 scoring + inversion MLP), SPMD across 8 NeuronCores.

Self-contained: hardcodes all shapes; only depends on the runtime environment
(/opt/trn_rl_repo concourse stack + numpy/ml_dtypes).
"""
import sys

if "/opt/trn_rl_repo" not in sys.path:
    sys.path.insert(0, "/opt/trn_rl_repo")

import numpy as np
import ml_dtypes

from concourse import bass, bacc, mybir, tile
from concourse.bass_utils import run_bass_kernel_spmd
from concourse.masks import make_identity

AF = mybir.ActivationFunctionType
ALU = mybir.AluOpType

P = 128
NEG = -1e9
BIGM = 32768.0

FULL_CFG = dict(N=12288, E=196608, H=128, Z=128, NC=8, CHUNK=512, NDEPTH=64)


# --------------------------------------------------------------------------
# Host-side index prep (sharding): pure indexing / counting, no FLOPs.
# --------------------------------------------------------------------------

def host_prep(inputs, cfg):
    N, E, H, Z, NC = cfg["N"], cfg["E"], cfg["H"], cfg["Z"], cfg["NC"]
    CHUNK, ND = cfg["CHUNK"], cfg["NDEPTH"]
    RPC = N // NC
    NT = RPC // P
    NCH = N // CHUNK

    x = np.asarray(inputs["x"], np.float32)
    depth = np.asarray(inputs["node_depth"], np.int32)
    ei = np.asarray(inputs["edge_index"], np.int32)
    src = np.concatenate([ei[0], np.arange(N, dtype=np.int32)])
    dst = np.concatenate([ei[1], np.arange(N, dtype=np.int32)])
    deg = np.bincount(dst, minlength=N).astype(np.float32)  # >= 1 (self loops)

    # order edges by dst, split per (core, tile)
    order = np.argsort(dst, kind="stable")
    s_s, d_s = src[order], dst[order]
    tile_of = d_s // P  # global dst tile id 0..N/P-1
    # counts per global tile
    tcnt = np.bincount(tile_of, minlength=N // P)
    NEPT_CH = int(np.ceil(tcnt.max() / P))  # chunks per tile (uniform)
    TOTCH = NT * NEPT_CH
    tstart = np.concatenate([[0], np.cumsum(tcnt)])

    per_core = []
    for c in range(NC):
        xsrc = np.zeros((TOTCH * P, 2), np.float32)
        degs = np.ones((TOTCH * P,), np.float32)
        dloc = np.full((TOTCH * P,), 255.0, np.float32)
        sidx = np.zeros((TOTCH * P,), np.int32)
        for t in range(NT):
            g = c * NT + t  # global tile
            e0, e1 = tstart[g], tstart[g + 1]
            n = e1 - e0
            o = t * NEPT_CH * P
            xsrc[o:o + n] = x[s_s[e0:e1]]
            degs[o:o + n] = deg[s_s[e0:e1]]
            dloc[o:o + n] = (d_s[e0:e1] - g * P).astype(np.float32)
            sidx[o:o + n] = s_s[e0:e1]

        # p-major [P, TOTCH, k] layouts (partition = within-chunk edge slot)
        def pmaj(a):
            a = a.reshape(TOTCH, P, -1)
            return np.ascontiguousarray(np.transpose(a, (1, 0, 2)))

        rows = np.arange(c * RPC, (c + 1) * RPC)
        dv = depth[rows]
        deg_own = deg[rows].reshape(NT, P).T.copy()  # [P, NT]

        # depth-indicator mask factors (bf16, exact for 0/1/BIGM)
        tind = np.zeros((1 + ND, NT * P), np.float32)
        tind[0, :] = -BIGM
        for l in range(ND):
            tind[1 + l, :] = BIGM * (dv > l)

        # fixup data for rows with <2 valid candidates
        cnt_d = np.bincount(depth, minlength=ND)
        cum = np.concatenate([[0], np.cumsum(cnt_d)])  # cum[v] = #depth< v
        pv = cum[dv]
        first_inv = np.zeros((ND,), np.int64)
        second_inv = np.zeros((ND,), np.int64)
        for l in range(ND):
            cands = np.nonzero(depth >= l)[0][:2]
            first_inv[l] = cands[0] if len(cands) > 0 else 0
            second_inv[l] = cands[1] if len(cands) > 1 else 1
        g0 = (pv == 0)
        g1 = (pv <= 1)
        f0 = np.where(g0, first_inv[dv], 0).astype(np.float32)
        # slot1 fix: for pv==0 -> second invalid; pv==1 -> first invalid
        f1 = np.where(g0, second_inv[dv], first_inv[dv]).astype(np.float32)

        def pm1(a, dt):  # [RPC] -> [P, NT]
            return np.ascontiguousarray(a.reshape(NT, P).T).astype(dt)

        per_core.append(dict(
            xsrc=pmaj(xsrc).astype(np.float32),
            degsrc=pmaj(degs)[:, :, 0].astype(np.float32),
            dstloc=pmaj(dloc)[:, :, 0].astype(np.float32),
            srcidx=pmaj(sidx)[:, :, 0].astype(np.int32),
            deg_own=deg_own.astype(np.float32),
            tind=tind.astype(ml_dtypes.bfloat16),
            g0=pm1(g0, np.uint32), g1=pm1(g1, np.uint32),
            f0=pm1(f0, np.float32), f1=pm1(f1, np.float32),
        ))

    sind = np.zeros((1 + ND, N), np.float32)
    sind[0, :] = 1.0
    for l in range(ND):
        sind[1 + l, :] = (depth == l)
    sind = sind.astype(ml_dtypes.bfloat16)

    shared = dict(
        sind=sind,
        W1=np.asarray(inputs["W1"], np.float32),
        W2=np.asarray(inputs["W2"], np.float32),
        Wm1a=np.asarray(inputs["Wm1"][:H], np.float32),
        Wm1b=np.asarray(inputs["Wm1"][H:], np.float32),
        Wm2=np.asarray(inputs["Wm2"], np.float32),
        Ws=np.asarray(inputs["Ws"], np.float32),
        Wt=np.asarray(inputs["Wt"], np.float32),
        Wi1a=np.asarray(inputs["Wi1"][:H], np.float32),
        Wi1b=np.asarray(inputs["Wi1"][H:2 * H], np.float32),
        Wi1c=np.asarray(inputs["Wi1"][2 * H:], np.float32),
        Wi2=np.asarray(inputs["Wi2"], np.float32),
        b1=np.asarray(inputs["b1"], np.float32).reshape(H, 1),
        b2=np.asarray(inputs["b2"], np.float32).reshape(H, 1),
        bm1=np.asarray(inputs["bm1"], np.float32).reshape(H, 1),
        bm2=np.asarray(inputs["bm2"], np.float32).reshape(H, 1),
        bi1=np.asarray(inputs["bi1"], np.float32).reshape(H, 1),
        bi2=np.asarray(inputs["bi2"], np.float32).reshape(1, 1),
        z=np.asarray(inputs["z"], np.float32).reshape(Z, 1),
    )

    in_maps = []
    for c in range(NC):
        m = dict(shared)
        m.update(per_core[c])
        in_maps.append(m)

    meta = dict(NEPT_CH=NEPT_CH, TOTCH=TOTCH, RPC=RPC, NT=NT, NCH=NCH)
    return in_maps, meta


# --------------------------------------------------------------------------
# Device program (same graph on all cores; per-core data via in_maps)
# --------------------------------------------------------------------------

def build_program(cfg, meta, nc=None, io=None):
    """Build the SPMD program. If io is given (sim tests), use those APs
    instead of declaring dram parameters."""
    N, E, H, Z, NC = cfg["N"], cfg["E"], cfg["H"], cfg["Z"], cfg["NC"]
    CHUNK, ND = cfg["CHUNK"], cfg["NDEPTH"]
    NEPT_CH, TOTCH, RPC, NT, NCH = (meta["NEPT_CH"], meta["TOTCH"],
                                    meta["RPC"], meta["NT"], meta["NCH"])
    FP = mybir.dt.float32
    BF = mybir.dt.bfloat16
    I32 = mybir.dt.int32
    U32 = mybir.dt.uint32

    own_nc = nc is None
    if own_nc:
        nc = bacc.Bacc()

    def param(name, shape, dtype, out=False):
        if io is not None:
            return io[name]
        return nc.declare_dram_parameter(name, list(shape), dtype,
                                         isOutput=out)

    xsrc_d = param("xsrc", (P, TOTCH, 2), FP)
    degsrc_d = param("degsrc", (P, TOTCH), FP)
    dstloc_d = param("dstloc", (P, TOTCH), FP)
    srcidx_d = param("srcidx", (P, TOTCH), I32)
    deg_own_d = param("deg_own", (P, NT), FP)
    tind_d = param("tind", (1 + ND, NT * P), BF)
    sind_d = param("sind", (1 + ND, N), BF)
    g0_d = param("g0", (P, NT), U32)
    g1_d = param("g1", (P, NT), U32)
    f0_d = param("f0", (P, NT), FP)
    f1_d = param("f1", (P, NT), FP)
    W1_d = param("W1", (2, H), FP)
    W2_d = param("W2", (H, H), FP)
    Wm1a_d = param("Wm1a", (H, H), FP)
    Wm1b_d = param("Wm1b", (Z, H), FP)
    Wm2_d = param("Wm2", (H, H), FP)
    Ws_d = param("Ws", (H, H), FP)
    Wt_d = param("Wt", (H, H), FP)
    Wi1a_d = param("Wi1a", (H, H), FP)
    Wi1b_d = param("Wi1b", (H, H), FP)
    Wi1c_d = param("Wi1c", (Z, H), FP)
    Wi2_d = param("Wi2", (H, 1), FP)
    b1_d = param("b1", (H, 1), FP)
    b2_d = param("b2", (H, 1), FP)
    bm1_d = param("bm1", (H, 1), FP)
    bm2_d = param("bm2", (H, 1), FP)
    bi1_d = param("bi1", (H, 1), FP)
    bi2_d = param("bi2", (1, 1), FP)
    z_d = param("z", (Z, 1), FP)
    out_d = param("out", (RPC, 6), FP, out=True)

    with tile.TileContext(nc) as tc:
        _build_body(tc, cfg, meta, locals())

    if own_nc:
        nc.compile()
        return nc
    return None


def build_into(tc, cfg, meta, io):
    """Sim-test path: io is a dict name->AP (including 'out')."""
    d = {name + "_d": ap for name, ap in io.items()}
    _build_body(tc, cfg, meta, d)


def _build_body(tc, cfg, meta, d):
    nc = tc.nc
    N, H, Z, NC = cfg["N"], cfg["H"], cfg["Z"], cfg["NC"]
    CHUNK, ND = cfg["CHUNK"], cfg["NDEPTH"]
    NEPT_CH, TOTCH, RPC, NT, NCH = (meta["NEPT_CH"], meta["TOTCH"],
                                    meta["RPC"], meta["NT"], meta["NCH"])
    FP = mybir.dt.float32
    BF = mybir.dt.bfloat16
    I32 = mybir.dt.int32
    U32 = mybir.dt.uint32
    CPT = CHUNK // P  # not used

    ctx = tc

    with tc.tile_pool(name="dram", bufs=1, space="DRAM") as dram, \
         tc.tile_pool(name="const", bufs=1) as cpool:

        # ---------- DRAM internals ----------
        h1p_own = dram.tile([RPC, H], FP)
        h1p_full = dram.tile([N, H], FP, addr_space="Shared")
        hT_bounce = dram.tile([H, RPC], FP)
        hT_stack = dram.tile([NC * H, RPC], FP, addr_space="Shared")
        hnm_own = dram.tile([RPC, H], FP)
        hnm_full = dram.tile([N, H], FP, addr_space="Shared")

        # ---------- constants / weights in SBUF ----------
        ident = cpool.tile([P, P], FP)
        make_identity(nc, ident[:])
        iota_i = cpool.tile([P, P], I32)
        nc.gpsimd.iota(iota_i[:], pattern=[[1, P]], base=0,
                       channel_multiplier=0)
        iota_f = cpool.tile([P, P], FP)
        nc.vector.tensor_copy(iota_f[:], iota_i[:])

        def load(name, shape, dtype=FP):
            t = cpool.tile(list(shape), dtype, name=name)
            nc.sync.dma_start(t[:], d[name + "_d"][:])
            return t

        W1 = load("W1", (2, H))
        W2 = load("W2", (H, H))
        Wm1a = load("Wm1a", (H, H))
        Wm1b = load("Wm1b", (Z, H))
        Wm2 = load("Wm2", (H, H))
        Ws = load("Ws", (H, H))
        Wt = load("Wt", (H, H))
        Wi1a = load("Wi1a", (H, H))
        Wi1b = load("Wi1b", (H, H))
        Wi1c = load("Wi1c", (Z, H))
        Wi2 = load("Wi2", (H, 1))
        b1 = load("b1", (H, 1))
        b2 = load("b2", (H, 1))
        bm1 = load("bm1", (H, 1))
        bm2 = load("bm2", (H, 1))
        bi1 = load("bi1", (H, 1))
        bi2 = load("bi2", (1, 1))
        zc = load("z", (Z, 1))
        tind = load("tind", (1 + ND, NT * P), BF)
        sind = load("sind", (1 + ND, N), BF)
        g0c = load("g0", (P, NT), U32)
        g1c = load("g1", (P, NT), U32)
        f0c = load("f0", (P, NT), FP)
        f1c = load("f1", (P, NT), FP)
        degown = load("deg_own", (P, NT), FP)

        negtile = cpool.tile([P, 1], FP)
        nc.vector.memset(negtile[:], NEG)

        # matmul instructions can carry at most one semaphore wait; a full
        # barrier here gives every preamble-loaded weight a single covered
        # provenance before any PE instruction runs.
        tc.strict_bb_all_engine_barrier()

        # fold z into mlp biases: biasm = bm1 + Wm1b.T @ z ; biasi = bi1 + ...
        with tc.tile_pool(name="ps0", bufs=1, space="PSUM") as ps0:
            bm_ps = ps0.tile([H, 1], FP, space="PSUM")
            nc.tensor.matmul(bm_ps[:], lhsT=Wm1b[:], rhs=zc[:], start=True,
                             stop=True)
            biasm = cpool.tile([H, 1], FP)
            nc.scalar.activation(biasm[:], bm_ps[:], AF.Identity,
                                 bias=bm1[:, :1])
            bi_ps = ps0.tile([H, 1], FP, space="PSUM")
            nc.tensor.matmul(bi_ps[:], lhsT=Wi1c[:], rhs=zc[:], start=True,
                             stop=True)
            biasi = cpool.tile([H, 1], FP)
            nc.scalar.activation(biasi[:], bi_ps[:], AF.Identity,
                                 bias=bi1[:, :1])

        # invsqrt of own dst degrees [P, NT]
        invd = cpool.tile([P, NT], FP)
        nc.vector.reciprocal(invd[:], degown[:])
        nc.scalar.activation(invd[:], invd[:], AF.Sqrt)

        # ---------------- Phase A: GCN layer 1 ----------------
        with tc.tile_pool(name="edges", bufs=1) as epool:
            xsrc = epool.tile([P, TOTCH, 2], FP)
            nc.sync.dma_start(xsrc[:], d["xsrc_d"][:])
            degsrc = epool.tile([P, TOTCH], FP)
            nc.sync.dma_start(degsrc[:], d["degsrc_d"][:])
            dstloc = epool.tile([P, TOTCH], FP)
            nc.sync.dma_start(dstloc[:], d["dstloc_d"][:])
            srcidx = epool.tile([P, TOTCH], I32)
            nc.sync.dma_start(srcidx[:], d["srcidx_d"][:])

            invsrc = epool.tile([P, TOTCH], FP)
            nc.vector.reciprocal(invsrc[:], degsrc[:])
            nc.scalar.activation(invsrc[:], invsrc[:], AF.Sqrt)
            g1v = epool.tile([P, TOTCH, 2], FP)
            nc.vector.tensor_tensor(out=g1v[:, :, 0], in0=xsrc[:, :, 0],
                                    in1=invsrc[:], op=ALU.mult)
            nc.vector.tensor_tensor(out=g1v[:, :, 1], in0=xsrc[:, :, 1],
                                    in1=invsrc[:], op=ALU.mult)

            with tc.tile_pool(name="l1", bufs=3) as l1p, \
                 tc.tile_pool(name="l1ps", bufs=1, space="PSUM") as l1ps, \
                 tc.tile_pool(name="l1acc", bufs=2, space="PSUM") as l1acc:
                for t in range(NT):
                    agg1_ps = l1acc.tile([P, 2], FP, space="PSUM",
                                         tag="agg1")
                    for ci in range(NEPT_CH):
                        c = t * NEPT_CH + ci
                        oh = l1p.tile([P, P], FP, tag="oh1")
                        nc.vector.tensor_scalar(
                            out=oh[:], in0=iota_f[:],
                            scalar1=dstloc[:, c:c + 1], scalar2=None,
                            op0=ALU.is_equal)
                        nc.tensor.matmul(agg1_ps[:], lhsT=oh[:],
                                         rhs=g1v[:, c, :],
                                         start=(ci == 0),
                                         stop=(ci == NEPT_CH - 1))
                    agg1 = l1p.tile([P, 2], FP, tag="agg1s")
                    nc.scalar.activation(agg1[:], agg1_ps[:], AF.Copy,
                                         scale=invd[:, t:t + 1])
                    agg1T_ps = l1ps.tile([2, P], FP, space="PSUM",
                                         tag="a1T")
                    nc.tensor.transpose(agg1T_ps[:], agg1[:], ident[:])
                    agg1T = l1p.tile([2, P], FP, tag="a1Ts")
                    nc.scalar.copy(agg1T[:], agg1T_ps[:])
                    h1T_ps = l1ps.tile([H, P], FP, space="PSUM", tag="h1T")
                    nc.tensor.matmul(h1T_ps[:], lhsT=W1[:], rhs=agg1T[:],
                                     start=True, stop=True)
                    h1T = l1p.tile([H, P], FP, tag="h1Ts")
                    nc.scalar.activation(h1T[:], h1T_ps[:], AF.Relu,
                                         bias=b1[:, :1])
                    h1nm_ps = l1ps.tile([P, H], FP, space="PSUM", tag="h1nm")
                    nc.tensor.transpose(h1nm_ps[:], h1T[:], ident[:])
                    h1p = l1p.tile([P, H], FP, tag="h1ps")
                    nc.scalar.activation(h1p[:], h1nm_ps[:], AF.Copy,
                                         scale=invd[:, t:t + 1])
                    nc.sync.dma_start(h1p_own[t * P:(t + 1) * P, :], h1p[:])

            # all-gather h1' (scaled) node-major
            nc.gpsimd.collective_compute(
                "AllGather", ALU.bypass,
                replica_groups=[list(range(NC))],
                ins=[h1p_own.opt()], outs=[h1p_full.opt()])

            # ---------------- Phase A: GCN layer 2 + node MLP ----------------
            hTown = cpool.tile([H, RPC], FP)
            with tc.tile_pool(name="l2", bufs=3) as l2p, \
                 tc.tile_pool(name="l2g", bufs=4) as l2g, \
                 tc.tile_pool(name="l2ps", bufs=1, space="PSUM") as l2ps, \
                 tc.tile_pool(name="l2acc", bufs=2, space="PSUM") as l2acc:
                for t in range(NT):
                    agg2_ps = l2acc.tile([P, H], FP, space="PSUM", tag="agg2")
                    g2all = l2g.tile([P, NEPT_CH, H], FP, tag="g2all")
                    nc.gpsimd.indirect_dma_start(
                        out=g2all[:], out_offset=None, in_=h1p_full[:],
                        in_offset=bass.IndirectOffsetOnAxis(
                            ap=srcidx[:, t * NEPT_CH:(t + 1) * NEPT_CH],
                            axis=0))
                    # single-provenance touch so chunk matmuls carry one wait
                    g2t = l2g.tile([P, NEPT_CH, H], FP, tag="g2t")
                    nc.vector.tensor_copy(g2t[:], g2all[:])
                    for ci in range(NEPT_CH):
                        c = t * NEPT_CH + ci
                        oh = l2p.tile([P, P], FP, tag="oh2")
                        nc.vector.tensor_scalar(
                            out=oh[:], in0=iota_f[:],
                            scalar1=dstloc[:, c:c + 1], scalar2=None,
                            op0=ALU.is_equal)
                        nc.tensor.matmul(agg2_ps[:], lhsT=oh[:],
                                         rhs=g2t[:, ci, :],
                                         start=(ci == 0),
                                         stop=(ci == NEPT_CH - 1))
                    agg2 = l2p.tile([P, H], FP, tag="agg2s")
                    nc.scalar.activation(agg2[:], agg2_ps[:], AF.Copy,
                                         scale=invd[:, t:t + 1])
                    agg2T_ps = l2ps.tile([H, P], FP, space="PSUM", tag="a2T")
                    nc.tensor.transpose(agg2T_ps[:], agg2[:], ident[:])
                    agg2T = l2p.tile([H, P], FP, tag="a2Ts")
                    nc.scalar.copy(agg2T[:], agg2T_ps[:])
                    h2T_ps = l2ps.tile([H, P], FP, space="PSUM", tag="h2T")
                    nc.tensor.matmul(h2T_ps[:], lhsT=W2[:], rhs=agg2T[:],
                                     start=True, stop=True)
                    h2T = l2p.tile([H, P], FP, tag="h2Ts")
                    nc.scalar.activation(h2T[:], h2T_ps[:], AF.Relu,
                                         bias=b2[:, :1])
                    hmT_ps = l2ps.tile([H, P], FP, space="PSUM", tag="hmT")
                    nc.tensor.matmul(hmT_ps[:], lhsT=Wm1a[:], rhs=h2T[:],
                                     start=True, stop=True)
                    hmT = l2p.tile([H, P], FP, tag="hmTs")
                    nc.scalar.activation(hmT[:], hmT_ps[:], AF.Relu,
                                         bias=biasm[:, :1])
                    hT_ps = l2ps.tile([H, P], FP, space="PSUM", tag="hT")
                    nc.tensor.matmul(hT_ps[:], lhsT=Wm2[:], rhs=hmT[:],
                                     start=True, stop=True)
                    nc.scalar.activation(hTown[:, t * P:(t + 1) * P],
                                         hT_ps[:], AF.Identity,
                                         bias=bm2[:, :1])
                    hnm_ps = l2ps.tile([P, H], FP, space="PSUM", tag="hnm")
                    nc.tensor.transpose(hnm_ps[:],
                                        hTown[:, t * P:(t + 1) * P],
                                        ident[:])
                    hnm = l2p.tile([P, H], FP, tag="hnms")
                    nc.scalar.copy(hnm[:], hnm_ps[:])
                    nc.sync.dma_start(hnm_own[t * P:(t + 1) * P, :], hnm[:])

        # all-gather hT blocks and node-major h
        nc.sync.dma_start(hT_bounce[:], hTown[:])
        nc.gpsimd.collective_compute(
            "AllGather", ALU.bypass, replica_groups=[list(range(NC))],
            ins=[hT_bounce.opt()], outs=[hT_stack.opt()])
        nc.gpsimd.collective_compute(
            "AllGather", ALU.bypass, replica_groups=[list(range(NC))],
            ins=[hnm_own.opt()], outs=[hnm_full.opt()])

        # ---------------- Phase B: S_T / T_T ----------------
        S_T = cpool.tile([H, N], FP)
        T_T = cpool.tile([H, RPC], FP)
        with tc.tile_pool(name="hTfull", bufs=1) as hfp, \
             tc.tile_pool(name="stps", bufs=4, space="PSUM") as stps:
            hT_full = hfp.tile([H, N], FP)
            for c in range(NC):
                nc.sync.dma_start(hT_full[:, c * RPC:(c + 1) * RPC],
                                  hT_stack[c * H:(c + 1) * H, :])
            for j in range(NCH):
                s_ps = stps.tile([P, CHUNK], FP, space="PSUM", tag="sps")
                nc.tensor.matmul(s_ps[:], lhsT=Ws[:],
                                 rhs=hT_full[:, j * CHUNK:(j + 1) * CHUNK],
                                 start=True, stop=True)
                nc.scalar.copy(S_T[:, j * CHUNK:(j + 1) * CHUNK], s_ps[:])
            for j in range(RPC // CHUNK if RPC >= CHUNK else 1):
                w = min(CHUNK, RPC)
                s_ps = stps.tile([P, w], FP, space="PSUM", tag="tps")
                nc.tensor.matmul(s_ps[:], lhsT=Wt[:],
                                 rhs=hTown[:, j * w:(j + 1) * w],
                                 start=True, stop=True)
                nc.scalar.copy(T_T[:, j * w:(j + 1) * w], s_ps[:])

        # ---------------- Phase C: scores + top2 + inv mlp ----------------
        with tc.tile_pool(name="scores", bufs=2) as scp, \
             tc.tile_pool(name="cps", bufs=3, space="PSUM") as cps, \
             tc.tile_pool(name="csml", bufs=3) as csm, \
             tc.tile_pool(name="ips", bufs=1, space="PSUM") as ips:
            for t in range(NT):
                sc = scp.tile([P, N], FP, tag="sc")
                for j in range(NCH):
                    st_ps = cps.tile([P, CHUNK], FP, space="PSUM", tag="st")
                    nc.tensor.matmul(
                        st_ps[:], lhsT=T_T[:, t * P:(t + 1) * P],
                        rhs=S_T[:, j * CHUNK:(j + 1) * CHUNK],
                        start=True, stop=False)
                    nc.tensor.matmul(
                        st_ps[:], lhsT=tind[:, t * P:(t + 1) * P],
                        rhs=sind[:, j * CHUNK:(j + 1) * CHUNK],
                        start=False, stop=True)
                    nc.scalar.copy(sc[:, j * CHUNK:(j + 1) * CHUNK],
                                   st_ps[:])
                maxv = csm.tile([P, 8], FP, tag="maxv")
                nc.vector.max(out=maxv[:], in_=sc[:])
                maxi = csm.tile([P, 8], U32, tag="maxi")
                nc.vector.max_index(out=maxi[:], in_max=maxv[:],
                                    in_values=sc[:])
                vals2 = csm.tile([P, 2], FP, tag="vals2")
                nc.vector.tensor_copy(vals2[:], maxv[:, 0:2])
                idxf = csm.tile([P, 2], FP, tag="idxf")
                nc.vector.tensor_copy(idxf[:], maxi[:, 0:2])
                # fixups for rows with <2 valid candidates
                nc.vector.copy_predicated(
                    vals2[:, 0:1], g0c[:, t:t + 1], negtile[:])
                nc.vector.copy_predicated(
                    vals2[:, 1:2], g1c[:, t:t + 1], negtile[:])
                nc.vector.copy_predicated(
                    idxf[:, 0:1], g0c[:, t:t + 1], f0c[:, t:t + 1])
                nc.vector.copy_predicated(
                    idxf[:, 1:2], g1c[:, t:t + 1], f1c[:, t:t + 1])
                nc.sync.dma_start(d["out_d"][t * P:(t + 1) * P, 0:2],
                                  vals2[:])
                nc.sync.dma_start(d["out_d"][t * P:(t + 1) * P, 2:4],
                                  idxf[:])
                for k in range(2):
                    idxi = csm.tile([P, 1], I32, tag="idxi")
                    nc.vector.tensor_copy(idxi[:], idxf[:, k:k + 1])
                    hu = csm.tile([P, H], FP, tag="hu")
                    nc.gpsimd.indirect_dma_start(
                        out=hu[:], out_offset=None, in_=hnm_full[:],
                        in_offset=bass.IndirectOffsetOnAxis(
                            ap=idxi[:, :1], axis=0))
                    huT_ps = ips.tile([H, P], FP, space="PSUM", tag="huT")
                    nc.tensor.transpose(huT_ps[:], hu[:], ident[:])
                    huT = csm.tile([H, P], FP, tag="huTs")
                    nc.scalar.copy(huT[:], huT_ps[:])
                    mid_ps = ips.tile([H, P], FP, space="PSUM", tag="mid")
                    nc.tensor.matmul(mid_ps[:], lhsT=Wi1a[:], rhs=huT[:],
                                     start=True, stop=False)
                    nc.tensor.matmul(mid_ps[:], lhsT=Wi1b[:],
                                     rhs=hTown[:, t * P:(t + 1) * P],
                                     start=False, stop=True)
                    mid = csm.tile([H, P], FP, tag="mids")
                    nc.scalar.activation(mid[:], mid_ps[:], AF.Relu,
                                         bias=biasi[:, :1])
                    log_ps = ips.tile([1, P], FP, space="PSUM", tag="logp")
                    nc.tensor.matmul(log_ps[:], lhsT=Wi2[:], rhs=mid[:],
                                     start=True, stop=True)
                    prob = csm.tile([1, P], FP, tag="prob")
                    nc.scalar.activation(prob[:], log_ps[:], AF.Sigmoid,
                                         bias=bi2[:, :1])
                    nc.sync.dma_start(
                        d["out_d"][t * P:(t + 1) * P,
                                   4 + k:5 + k].rearrange("a b -> b a"),
                        prob[:])


# --------------------------------------------------------------------------
# Entry point
# --------------------------------------------------------------------------

_CACHE = {}


def kernel(**inputs):
    cfg = FULL_CFG
    in_maps, meta = host_prep(inputs, cfg)
    key = (cfg["N"], meta["NEPT_CH"])
    if key not in _CACHE:
        _CACHE[key] = build_program(cfg, meta)
    nc = _CACHE[key]
    res = run_bass_kernel_spmd(nc, in_maps, core_ids=list(range(cfg["NC"])))
    global _LAST_EXEC_NS
    _LAST_EXEC_NS = getattr(res, "exec_time_ns", None)
    outs = [res.results[c]["out"] for c in range(cfg["NC"])]
    out = np.concatenate(outs, axis=0)  # [N, 6]

    vals = out[:, 0:2].astype(np.float32)
    idx = np.rint(out[:, 2:4]).astype(np.int32)
    inv_prob = out[:, 4:6].astype(np.float32)
    nt = np.asarray(inputs["node_type"], np.int32)
    nd = np.asarray(inputs["node_depth"], np.int32)
    tv = (nt != 0) & (nd > 0) & (vals[:, 0] > NEG / 2)
    edge_valid = np.stack([tv, tv & (nt == 2)], axis=1)
    return inv_prob, vals, idx, edge_valid
